# revision 12
# baseline (speedup 1.0000x reference)
"""Local (sliding-window) attention, sharded over 8 NeuronCores by (batch, head).

Core c owns batch c//4 and heads 4*(c%4) .. 4*(c%4)+3. Each core:
  - projects q,k (fp32 matmuls, transposed layout [dims, seq]) and v (bf16)
  - computes banded scores (window +-128), softmax in fp32
  - writes its [4, 2048, 2048] fp32 slab of attn_weights (zeros off-band)
  - computes its partial output projection (bf16); host sums partials + bo.
"""
import sys

sys.path.insert(0, "/opt/trn_rl_repo")

import numpy as np
import ml_dtypes

B, S, D = 2, 2048, 1024
H, DH = 16, 64
WIN = 128            # half window; valid iff |i-j| <= 128
HPC = 4              # heads per core
N_CORES = 8
QB = S // 128        # 16 q-blocks of 128 rows
KW = 384             # key window per q-block (3 x 128, aligned)
PAD = 128            # key padding each side
SP = S + 2 * PAD     # padded key axis = 2304
NEG = -8.0e9         # additive mask before the 1/8 scale -> -1e9 after

_CACHE = {}


def _build():
    import concourse.tile as tile
    from concourse import bacc, mybir
    from concourse.masks import make_identity

    f32 = mybir.dt.float32
    bf16 = mybir.dt.bfloat16

    nc = bacc.Bacc("TRN2", target_bir_lowering=False, debug=False,
                   enable_asserts=False, num_devices=N_CORES)

    qt = nc.dram_tensor("qt", [D, S], f32, kind="ExternalInput").ap()
    kt = nc.dram_tensor("kt", [D, S], f32, kind="ExternalInput").ap()
    vt = nc.dram_tensor("vt", [D, S], bf16, kind="ExternalInput").ap()
    wq = nc.dram_tensor("wq", [D, HPC * DH], f32, kind="ExternalInput").ap()
    wk = nc.dram_tensor("wk", [D, HPC * DH], f32, kind="ExternalInput").ap()
    wv = nc.dram_tensor("wv", [D, HPC * DH], bf16, kind="ExternalInput").ap()
    wo = nc.dram_tensor("wo", [HPC * DH, D], bf16, kind="ExternalInput").ap()
    bq = nc.dram_tensor("bq", [128, 2], f32, kind="ExternalInput").ap()
    bk = nc.dram_tensor("bk", [128, 2], f32, kind="ExternalInput").ap()
    bvb = nc.dram_tensor("bvb", [128, HPC * DH], f32, kind="ExternalInput").ap()
    msk = nc.dram_tensor("msk", [128, 3, KW], f32, kind="ExternalInput").ap()
    attn = nc.dram_tensor("attn", [HPC, S, S], f32, kind="ExternalOutput").ap()
    outp = nc.dram_tensor("outp", [S, D], f32, kind="ExternalOutput").ap()

    with tile.TileContext(nc) as tc:
        with tc.tile_pool(name="persist", bufs=1) as pp, \
             tc.tile_pool(name="ps", bufs=8, space="PSUM") as psp:
            # --- persistent tiles ---
            qT = pp.tile([128, 2, S], mybir.dt.float32r, tag="qT")          # [dim%128, dim//128, s]
            kT = pp.tile([128, 2, SP], mybir.dt.float32r, tag="kT")         # padded key axis
            vS = pp.tile([128, SP // 128, HPC * DH], bf16, tag="vS")  # [key%128, key//128, dim]
            cT = pp.tile([128, 2, S], bf16, tag="cT")         # contextT
            wo_sb = pp.tile([128, 2, D], bf16, tag="wo")
            msk_sb = pp.tile([128, 3, KW], f32, tag="msk")
            zero_sb = pp.tile([128, 1024], f32, tag="zero")
            ident = pp.tile([128, 128], f32, tag="ident")
            bq_sb = pp.tile([128, 2], f32, tag="bq")
            bk_sb = pp.tile([128, 2], f32, tag="bk")
            bvb_sb = pp.tile([128, HPC * DH], f32, tag="bvb")

            nc.gpsimd.memset(zero_sb[:], 0.0)
            make_identity(nc, ident[:])
            nc.sync.dma_start(msk_sb[:], msk.rearrange("p v w -> p v w"))
            nc.sync.dma_start(wo_sb[:], wo.rearrange("(ko ki) o -> ki ko o", ki=128))
            nc.sync.dma_start(bq_sb[:], bq[:])
            nc.sync.dma_start(bk_sb[:], bk[:])
            nc.sync.dma_start(bvb_sb[:], bvb[:])
            # pad regions of kT and vS must be zero
            nc.gpsimd.memset(kT[:, :, 0:PAD].bitcast(f32), 0.0)
            nc.gpsimd.memset(kT[:, :, S + PAD:SP].bitcast(f32), 0.0)
            nc.gpsimd.memset(vS[:, 0, :], 0.0)
            nc.gpsimd.memset(vS[:, SP // 128 - 1, :], 0.0)

            Ident = mybir.ActivationFunctionType.Identity
            Exp = mybir.ActivationFunctionType.Exp

            # --- phase A: projections ---
            with tc.tile_pool(name="pin", bufs=1) as pin, \
                 tc.tile_pool(name="pw", bufs=1) as pw:
                # v natural [keys, dims] (bf16), keys padded by one 128-tile each side
                bigv = pin.tile([128, 8, S], bf16, tag="big_v")
                nc.sync.dma_start(bigv[:], vt.rearrange("(ko ki) s -> ki ko s", ki=128))
                w16 = pw.tile([128, 8, HPC * DH], bf16, tag="w16")
                nc.sync.dma_start(w16[:], wv.rearrange("(ko ki) o -> ki ko o", ki=128))
                for kt_i in range(S // 128):
                    ps = psp.tile([128, 512], f32, tag="ps")
                    for ko in range(8):
                        nc.tensor.matmul(ps[:, :HPC * DH],
                                         bigv[:, ko, kt_i * 128:(kt_i + 1) * 128],
                                         w16[:, ko, :],
                                         start=(ko == 0), stop=(ko == 7))
                    nc.vector.tensor_add(vS[:, kt_i + 1, :], ps[:, :HPC * DH],
                                         bvb_sb[:])
                # stream the off-band zero-fills now: they ride the sync/HWDGE
                # queues while the q/k projections load via gpsimd queues
                for hl in range(HPC):
                    for qb in range(QB):
                        cs = qb * 128 - PAD
                        c0, c1 = max(0, cs), min(S, cs + KW)

                # qT = Wq^T @ Q^T   (fp32)
                big = pin.tile([128, 8, S], mybir.dt.float32r, tag="big_in")
                nc.gpsimd.dma_start(big[:], qt.rearrange("(ko ki) s -> ki ko s", ki=128))
                w32 = pw.tile([128, 8, HPC * DH], mybir.dt.float32r, tag="w32")
                nc.gpsimd.dma_start(w32[:], wq.rearrange("(ko ki) o -> ki ko o", ki=128))
                for mo in range(2):
                    for n4 in range(4):
                        ps = psp.tile([128, 512], f32, tag="ps")
                        for ko in range(8):
                            nc.tensor.matmul(
                                ps[:],
                                w32[:, ko, mo * 128:(mo + 1) * 128],
                                big[:, ko, n4 * 512:(n4 + 1) * 512],
                                start=(ko == 0), stop=(ko == 7))
                        nc.scalar.activation(qT[:, mo, n4 * 512:(n4 + 1) * 512],
                                             ps[:], Ident, bias=bq_sb[:, mo:mo + 1])

                # kT (fp32), into padded region [PAD : S+PAD]
                big = pin.tile([128, 8, S], mybir.dt.float32r, tag="big_in")
                nc.gpsimd.dma_start(big[:], kt.rearrange("(ko ki) s -> ki ko s", ki=128))
                w32 = pw.tile([128, 8, HPC * DH], mybir.dt.float32r, tag="w32")
                nc.gpsimd.dma_start(w32[:], wk.rearrange("(ko ki) o -> ki ko o", ki=128))
                for mo in range(2):
                    for n4 in range(4):
                        ps = psp.tile([128, 512], f32, tag="ps")
                        for ko in range(8):
                            nc.tensor.matmul(
                                ps[:],
                                w32[:, ko, mo * 128:(mo + 1) * 128],
                                big[:, ko, n4 * 512:(n4 + 1) * 512],
                                start=(ko == 0), stop=(ko == 7))
                        nc.scalar.activation(
                            kT[:, mo, PAD + n4 * 512:PAD + (n4 + 1) * 512],
                            ps[:], Ident, bias=bk_sb[:, mo:mo + 1])


            # --- phase B: banded attention ---
            with tc.tile_pool(name="pat", bufs=6) as pat:
                for hl in range(HPC):
                    p0 = (hl % 2) * 64
                    po = hl // 2
                    for qb in range(QB):
                        mv = 0 if qb == 0 else (2 if qb == QB - 1 else 1)
                        # scores: psum [128q, 384k] fp32
                        sps = psp.tile([128, 512], f32, tag="ps")
                        nc.tensor.matmul(
                            sps[:, :KW],
                            qT[p0:p0 + 64, po, qb * 128:(qb + 1) * 128],
                            kT[p0:p0 + 64, po, qb * 128:qb * 128 + KW],
                            start=True, stop=True)
                        ssb = pat.tile([128, KW], f32, tag="S")
                        nc.vector.tensor_add(ssb[:], sps[:, :KW], msk_sb[:, mv, :])
                        esb = pat.tile([128, KW], f32, tag="E")
                        rsum = pat.tile([128, 1], f32, tag="rsum")
                        nc.scalar.activation(esb[:], ssb[:], Exp, scale=0.125,
                                             accum_out=rsum[:])
                        rinv = pat.tile([128, 1], f32, tag="rinv")
                        nc.vector.reciprocal(rinv[:], rsum[:])
                        psb = pat.tile([128, KW], f32, tag="P")
                        nc.vector.tensor_scalar_mul(psb[:], esb[:], rinv[:])

                        # band write (clipped at edges) + zero fill off-band
                        cs = qb * 128 - PAD
                        c0, c1 = max(0, cs), min(S, cs + KW)
                        nc.sync.dma_start(
                            attn[hl, qb * 128:(qb + 1) * 128, c0:c1],
                            psb[:, c0 - cs:c1 - cs])

                        # transpose probs -> PT (bf16), context^T accumulation
                        cps = psp.tile([128, 128], f32, tag="ps")
                        for j in range(3):
                            tps = psp.tile([128, 128], f32, tag="ps")
                            nc.tensor.transpose(tps[:], psb[:, j * 128:(j + 1) * 128],
                                                ident[:])
                            ptb = pat.tile([128, 128], bf16, tag="PT")
                            nc.vector.tensor_copy(ptb[:], tps[:])
                            nc.tensor.matmul(
                                cps[p0:p0 + 64, :],
                                vS[:, qb + j, hl * DH:(hl + 1) * DH],
                                ptb[:],
                                start=(j == 0), stop=(j == 2),
                                tile_position=(0, p0))
                        nc.scalar.activation(cT[p0:p0 + 64, po, qb * 128:(qb + 1) * 128],
                                             cps[p0:p0 + 64, :], Ident)

            # --- phase C: partial output projection (bf16) ---
            with tc.tile_pool(name="pout", bufs=3) as pout:
                for ro in range(S // 128):
                    for no in range(2):
                        ps = psp.tile([128, 512], f32, tag="ps")
                        for ko in range(2):
                            nc.tensor.matmul(ps[:],
                                             cT[:, ko, ro * 128:(ro + 1) * 128],
                                             wo_sb[:, ko, no * 512:(no + 1) * 512],
                                             start=(ko == 0), stop=(ko == 1))
                        osb = pout.tile([128, 512], f32, tag="osb")
                        nc.vector.tensor_copy(osb[:], ps[:])
                        nc.sync.dma_start(outp[ro * 128:(ro + 1) * 128,
                                               no * 512:(no + 1) * 512], osb[:])

    nc.compile()
    return nc


def _get_nc():
    if "nc" not in _CACHE:
        _CACHE["nc"] = _build()
    return _CACHE["nc"]


def _host_mask():
    m = np.full((128, 3, KW), NEG, dtype=np.float32)
    for vi, qb in enumerate((0, 1, QB - 1)):
        for r in range(128):
            q = qb * 128 + r
            for w in range(KW):
                j = qb * 128 - PAD + w
                if abs(q - j) <= WIN and 0 <= j < S:
                    m[r, vi, w] = 0.0
    return m


def kernel(**inputs):
    from concourse import bass_utils

    Q = np.asarray(inputs["Q"], dtype=np.float32)
    K = np.asarray(inputs["K"], dtype=np.float32)
    V = np.asarray(inputs["V"], dtype=np.float32)
    Wq = np.asarray(inputs["Wq"], dtype=np.float32)
    Wk = np.asarray(inputs["Wk"], dtype=np.float32)
    Wv = np.asarray(inputs["Wv"], dtype=np.float32)
    Wo = np.asarray(inputs["Wo"], dtype=np.float32)
    bq = np.asarray(inputs["bq"], dtype=np.float32)
    bk = np.asarray(inputs["bk"], dtype=np.float32)
    bv = np.asarray(inputs["bv"], dtype=np.float32)
    bo = np.asarray(inputs["bo"], dtype=np.float32)

    nc = _get_nc()
    mask = _host_mask()
    bf = ml_dtypes.bfloat16

    in_maps = []
    for c in range(N_CORES):
        b = c // 4
        h0 = HPC * (c % 4)
        cols = slice(h0 * DH, (h0 + HPC) * DH)
        in_maps.append({
            "qt": np.ascontiguousarray(Q[b].T),
            "kt": np.ascontiguousarray(K[b].T),
            "vt": np.ascontiguousarray(V[b].T).astype(bf),
            "wq": np.ascontiguousarray(Wq[:, cols]),
            "wk": np.ascontiguousarray(Wk[:, cols]),
            "wv": np.ascontiguousarray(Wv[:, cols]).astype(bf),
            "wo": np.ascontiguousarray(Wo[cols, :]).astype(bf),
            "bq": np.ascontiguousarray(bq[cols].reshape(2, 128).T),
            "bk": np.ascontiguousarray(bk[cols].reshape(2, 128).T),
            "bvb": np.ascontiguousarray(
                np.broadcast_to(bv[cols], (128, HPC * DH))).astype(np.float32),
            "msk": np.ascontiguousarray(mask),
        })

    _CACHE["in_maps"] = in_maps
    res = bass_utils.run_bass_kernel_spmd(nc, in_maps,
                                          core_ids=list(range(N_CORES)))

    attn = np.empty((B, H, S, S), dtype=np.float32)
    out = np.zeros((B, S, D), dtype=np.float32)
    for c in range(N_CORES):
        b = c // 4
        h0 = HPC * (c % 4)
        r = res.results[c]
        attn[b, h0:h0 + HPC] = r["attn"]
        out[b] += r["outp"]
    out += bo
    return out, attn
